# revision 101
# baseline (speedup 1.0000x reference)
"""Trainium2 Bass kernel for nn_DynamicMLP (3-layer LIF spiking net, T=16).

Strategy (8 NeuronCores, data-parallel over batch):
  - Shard batch 1024 -> 8 x 128. Replicate weights. Zero cross-core comms.
  - TRANSPOSED layout: [hidden on partitions (128-chunks on free), batch on
    free dim]. Weights are the stationary matmul operand, activations the
    moving one; every layer's spikes come out exactly in the next layer's
    moving-operand layout, so there are NO DMA transposes anywhere.
  - LIF current state c lives in PSUM scaled by 2^t: C_t = sum 2^tau I_tau.
  - L0 (x @ W0): fp16 multi-term split, fp32-exact to ~2^-22 (the network is
    chaotic: >=20 significant bits needed on BOTH operands; measured):
      w0h.T@xh -> C0;  w0h.T@(2^11*xl) and (w0l*2^11).T@xh -> C0b,
    folded at 2^-(t+11) on release.
  - L1 (spikes @ W1): W1 decomposed into 6 signed radix-16 digits stored as
    exact fp8e4 planes (stationary); spikes as fp8e5 planes at 3 scales
    {2^t, 2^(t-8), 2^(t-16)} (moving). Matmuls run as fp8 DoubleRow pairs
    (2 digit products per instr at 0.5 cycles/row): 25% fewer PE cycles than
    the fp16 hi/lo split, ~21.5-bit effective weights (verified on device).
  - L2: fp16 hi/lo 2-term (digit planes for it don't fit SBUF).
  - Biases: one -b*2^e matmul injected into each C group at t=0; the +2b
    constant enters through the fused v-update (c_t = C*2^-t - b*2^-t + 2b).
    No per-step bias matmuls.
  - Fused LIF elementwise (v_t = v0^2 - u0 + c) split across DVE + ACT; the
    per-layer A = v0^2 - u0 + B2b is precomputed one step early off the
    critical path. Output accumulation (acc += v>vth) on DVE, not the PE.
"""
import sys

sys.path.insert(0, "/opt/trn_rl_repo")

import numpy as np
import ml_dtypes

import concourse.bacc as bacc
import concourse.tile as tile
from concourse import mybir
from concourse.bass_utils import run_bass_kernel_spmd

dt = mybir.dt
F16 = dt.float16
F32 = dt.float32
E4 = dt.float8e4
E5 = dt.float8e5
Alu = mybir.AluOpType
DR = mybir.MatmulPerfMode.DoubleRow

NCORES = 8
FULL = dict(T=16, IN=2048, H0=1024, H1=1024, OUT=512, BL=128)
NDIG = 6
EW = 4          # weight exponent for L1: W*2^EW in (-0.5, 0.5]
WFOLD = [4, 0, 4, 0, 4, 8]        # digit i stored as d_i * 2^-WFOLD[i]
KFOLD = [0, -8, -8, -16, -16, -16]  # spike plane scale exponents
# plane order in the sP tile: [s*2^t, s*2^(t-8), s*2^(t-16), s*2^(t-16)]
PLANE_OF_PAIR = [(0, 1), (1, 2), (2, 3)]  # pairs (d1,d2),(d3,d4),(d5,d6)

_BUILD_CACHE = {}


def build(T=16, IN=2048, H0=1024, H1=1024, OUT=512, BL=128):
    key = (T, IN, H0, H1, OUT, BL)
    if key in _BUILD_CACHE:
        return _BUILD_CACHE[key]
    KT0, KT1, KT2 = IN // 128, H0 // 128, H1 // 128
    MT0, MT1, MT2 = H0 // 128, H1 // 128, OUT // 128
    ZR = 512  # psum zero-region, fp32 elems (2KB)

    nc = bacc.Bacc("TRN2", target_bir_lowering=False, debug=False, num_devices=NCORES)

    # x hi/lo interleaved per 128-row group: rows (k,p,{hi,lo}) x BL cols —
    # keeps dram runs at 512B (no small-transfer DMA penalty), 1 DMA per tile
    xz_d = nc.dram_tensor("xz", [T, IN * 2, BL], F16, kind="ExternalInput")
    w0a_d = nc.dram_tensor("w0a", [IN, H0], F16, kind="ExternalInput")
    w0l_d = nc.dram_tensor("w0l", [IN, H0], F16, kind="ExternalInput")
    w1d_d = nc.dram_tensor("w1d", [H0, NDIG * H1], E4, kind="ExternalInput")
    woa_d = nc.dram_tensor("woa", [H1, OUT], F16, kind="ExternalInput")
    wol_d = nc.dram_tensor("wol", [H1, OUT], F16, kind="ExternalInput")
    br_d = {}
    for nm, h in (("br0", H0), ("br1", H1), ("br2", OUT)):
        br_d[nm] = nc.dram_tensor(nm, [2, h], F16, kind="ExternalInput")
    cst_d = {nm: nc.dram_tensor(nm, [2, 128], F16, kind="ExternalInput")
             for nm in ("cpos", "cneg0", "cneg1", "cneg2")}
    # out[p, (c,b)] = acc for out-neuron c*128+p, batch b (host de-permutes)
    out_d = nc.dram_tensor("out", [128, OUT // 128 * BL], F32,
                           kind="ExternalOutput")

    with tile.TileContext(nc) as tc:
        with tc.tile_pool(name="w", bufs=1) as wp, \
             tc.tile_pool(name="state", bufs=1) as sp, \
             tc.tile_pool(name="xs", bufs=2) as xp, \
             tc.tile_pool(name="spk", bufs=1) as kp, \
             tc.tile_pool(name="psum", bufs=1, space="PSUM") as pp:

            # ---- resident weights ----
            KH = KT0 // 2          # w0 split in 2 chunk-tiles for skew filler
            NX0 = KT0 // KH
            w_sb = {}
            for sfx in ("a", "l"):
                w_sb["w0" + sfx] = [
                    wp.tile([128, KH * H0], F16, tag=f"w0{sfx}{ci}",
                            name=f"w0{sfx}{ci}")
                    for ci in range(NX0)]
            w1d = wp.tile([128, KT1 * NDIG * H1], E4, tag="w1d", name="w1d")
            woa = wp.tile([128, KT2 * OUT], F16, tag="woa", name="woa")
            wol = wp.tile([128, KT2 * OUT], F16, tag="wol", name="wol")

            b_sb = {}
            for nm, h in (("br0", H0), ("br1", H1), ("br2", OUT)):
                b_sb[nm] = wp.tile([2, h], F16, tag=nm, name=nm)
            EL = {0: 0, 1: EW, 2: 0}   # per-layer PSUM weight exponent
            cpos = wp.tile([2, 128], F16, tag="cpos", name="cpos")
            cneg = {l: wp.tile([2, 128], F16, tag=f"cneg{l}", name=f"cneg{l}")
                    for l in (0, 1, 2)}

            # ---- states (free dim = (hidden-chunk, batch)) ----
            HS = {0: H0, 1: H1, 2: OUT}
            st = {}
            for l in (0, 1, 2):
                for nm in ("v0", "u0"):
                    st[(l, nm)] = sp.tile([128, HS[l]], F32, tag=f"{nm}{l}",
                                          name=f"{nm}{l}")
            vT = {0: sp.tile([128, H0], F32, tag="vT0", name="vT0"),
                  1: sp.tile([128, max(H1, OUT)], F32, tag="vT12", name="vT12")}
            vT[2] = vT[1]
            A_ = {0: sp.tile([128, H0], F32, tag="A0", name="A0"),
                  1: sp.tile([128, H1], F32, tag="A1", name="A1"),
                  2: sp.tile([128, OUT], F32, tag="A2", name="A2")}
            U_ = A_  # disjoint lifetimes: A dies at release, U born at post
            c021 = sp.tile([128, max(H0, H1)], F32, tag="c021")
            B2b = {0: sp.tile([128, H0], F32, tag="B2b0", name="B2b0"),
                   1: sp.tile([128, H1], F32, tag="B2b1", name="B2b1"),
                   2: sp.tile([128, OUT], F32, tag="B2b2", name="B2b2")}
            acc = sp.tile([128, OUT], F32, tag="acc", name="acc")
            C = {0: pp.tile([128, H0], F32, tag="C0", name="C0"),
                 1: pp.tile([128, H1], F32, tag="C1", name="C1"),
                 2: pp.tile([128, OUT], F32, tag="C2", name="C2")}
            C0b = pp.tile([128, H0], F32, tag="C0b", name="C0b")
            pB = pp.tile([128, 512], F32, tag="pB", name="pB")

            # ---- x loading ----
            x_pre = {}

            def load_x(t, cis=None):
                tiles = x_pre.setdefault(t, {})
                for ci in (cis if cis is not None else range(NX0)):
                    if ci in tiles:
                        continue
                    xz_t = xp.tile([128, KH * 2 * BL], F16, tag="xz",
                                   name=f"xz_t{t}_{ci}")
                    ks = ci * KH * 2 * 128
                    nc.sync.dma_start(
                        out=xz_t[:].rearrange("p (k two b) -> p k two b",
                                              two=2, b=BL),
                        in_=xz_d[t:t + 1, ks:ks + KH * 2 * 128].rearrange(
                            "o (k p two) b -> p (o k) two b", p=128, two=2))
                    tiles[ci] = xz_t

            def dma_w0(ci):
                for kk in range(KH):
                    k = ci * KH + kk
                    for sfx in ("a", "l"):
                        tl = w_sb["w0" + sfx][ci]
                        wd = w0a_d if sfx == "a" else w0l_d
                        nc.sync.dma_start(out=tl[:, kk * H0:(kk + 1) * H0],
                                          in_=wd[k * 128:(k + 1) * 128, :])

            def dma_w1d(ks):
                for k in ks:
                    nc.sync.dma_start(
                        out=w1d[:, k * NDIG * H1:(k + 1) * NDIG * H1],
                        in_=w1d_d[k * 128:(k + 1) * 128, :])

            def dma_wo():
                for k in range(KT2):
                    nc.sync.dma_start(out=woa[:, k * OUT:(k + 1) * OUT],
                                      in_=woa_d[k * 128:(k + 1) * 128, :])
                    nc.sync.dma_start(out=wol[:, k * OUT:(k + 1) * OUT],
                                      in_=wol_d[k * 128:(k + 1) * 128, :])

            # just-in-time DMA order (single serialized DMA resource)
            load_x(0, cis=(0,))
            dma_w0(0)
            nc.sync.dma_start(out=cpos[:], in_=cst_d["cpos"][:])
            for l in (0, 1, 2):
                nc.sync.dma_start(out=cneg[l][:], in_=cst_d[f"cneg{l}"][:])
            for nm in ("br0", "br1", "br2"):
                nc.sync.dma_start(out=b_sb[nm][:], in_=br_d[nm][:])
            dma_w1d(range(0, 2))
            load_x(0, cis=(1,))
            dma_w0(1)

            # init states + consts
            for l in (0, 1, 2):
                for nm in ("v0", "u0"):
                    nc.vector.memset(st[(l, nm)][:], 0.0)
            nc.vector.memset(c021[:], 0.021)
            nc.vector.memset(acc[:], 0.0)

            bias_of = {0: "br0", 1: "br1", 2: "br2"}

            def build_B2b():
                # B2b_l[p, (c,b)] = 2*b_l[c*128+p]: PE outer products, 4
                # chunks per pB fill, one batched copy per fill
                for l in (0, 1, 2):
                    for m0 in range(0, HS[l] // 128, 4):
                        mn = min(4, HS[l] // 128 - m0)
                        for j in range(mn):
                            m = m0 + j
                            nc.tensor.matmul(
                                pB[:, j * 128:(j + 1) * 128],
                                b_sb[bias_of[l]][:, m * 128:(m + 1) * 128],
                                cpos[:], start=True, stop=True,
                                skip_group_check=True)
                        nc.scalar.copy(
                            B2b[l][:, m0 * 128:(m0 + mn) * 128],
                            pB[:, :mn * 128])

            def inject_bias(l):
                # add -b*2^EL[l] into each C[l] 128-chunk at t=0
                for m in range(HS[l] // 128):
                    nc.tensor.matmul(
                        C[l][:, m * 128:(m + 1) * 128],
                        b_sb[bias_of[l]][:, m * 128:(m + 1) * 128],
                        cneg[l][:], start=False, stop=False,
                        skip_group_check=True)

            # ---- L0 matmuls (fp16 3-term; stationary = w0 chunks) ----
            def emit_L0(t, cis):
                load_x(t, cis=cis)
                tiles = x_pre[t]
                for ci in cis:
                    xz_t = tiles.pop(ci)
                    if not tiles:
                        x_pre.pop(t, None)
                    wa = w_sb["w0a"][ci]
                    wl = w_sb["w0l"][ci]
                    # C0 main terms first: the step's first matmuls only wait
                    # on the C0 release-stt, not the C0b one
                    for k in range(KH):
                        kg = ci * KH + k
                        ra = xz_t[:, (2 * k) * BL:(2 * k + 1) * BL]
                        for m in range(MT0):
                            first = (t == 0 and kg == 0 and
                                     (m * 128) % ZR == 0)
                            lwa = wa[:, k * H0 + m * 128:
                                     k * H0 + (m + 1) * 128]
                            nc.tensor.matmul(C[0][:, m * 128:(m + 1) * 128],
                                             lwa, ra, start=first, stop=False,
                                             skip_group_check=True)
                    for k in range(KH):
                        kg = ci * KH + k
                        ra = xz_t[:, (2 * k) * BL:(2 * k + 1) * BL]
                        rl = xz_t[:, (2 * k + 1) * BL:(2 * k + 2) * BL]
                        for m in range(MT0):
                            first = (t == 0 and kg == 0 and
                                     (m * 128) % ZR == 0)
                            last = (t == T - 1 and kg == KT0 - 1)
                            lwa = wa[:, k * H0 + m * 128:
                                     k * H0 + (m + 1) * 128]
                            lwl = wl[:, k * H0 + m * 128:
                                     k * H0 + (m + 1) * 128]
                            psb = C0b[:, m * 128:(m + 1) * 128]
                            nc.tensor.matmul(psb, lwa, rl, start=first,
                                             stop=False,
                                             skip_group_check=True)
                            nc.tensor.matmul(psb, lwl, ra, start=False,
                                             stop=last,
                                             skip_group_check=True)
                    if t == 0 and ci == NX0 - 1:
                        inject_bias(0)

            # ---- L1: fp8 DoubleRow digit matmuls ----
            def emit_dr(t):
                sP = sP_cur[0]
                for k in range(KT1):
                    for m in range(MT1):
                        for pi, (pa, pb_) in enumerate(PLANE_OF_PAIR):
                            first = (t == 0 and k == 0 and pi == 0 and
                                     (m * 128) % ZR == 0)
                            last = (t == T - 1 and k == KT1 - 1 and pi == 2)
                            base = (k * NDIG + 2 * pi) * H1
                            lhs = w1d[:, base:base + 2 * H1].rearrange(
                                "p (two h) -> p two h", two=2)[
                                :, :, m * 128:(m + 1) * 128]
                            rhs = sP[:, pa * H0:(pa + 2) * H0].rearrange(
                                "p (two h) -> p two h", two=2)[
                                :, :, k * 128:(k + 1) * 128]
                            nc.tensor.matmul(
                                C[1][:, m * 128:(m + 1) * 128], lhs, rhs,
                                start=first, stop=last, perf_mode=DR,
                                skip_group_check=True)
                if t == 0:
                    inject_bias(1)

            # ---- fused LIF elementwise (layout-agnostic) ----
            def lif_pre(l, t):
                """Off-path: A = v0*v0 - u0 + B2b (ACT square + 2 DVE ops)."""
                h = HS[l]
                A = A_[l][:, :h]
                v0, u0 = st[(l, "v0")], st[(l, "u0")]
                nc.scalar.square(A, v0[:])
                nc.vector.tensor_tensor(out=A, in0=A, in1=u0[:],
                                        op=Alu.subtract)
                nc.vector.tensor_tensor(out=A, in0=A, in1=B2b[l][:], op=Alu.add)

            def lif_release(l, t, halves=1):
                """DVE, reads PSUM: v = C*2^(-t-e) + A (+ C0b part for l=0).
                halves=2 on the last step (no filler work to hide the chain)."""
                h = HS[l]
                hh = h // halves
                for off in range(0, h, hh):
                    v = vT[l][:, off:off + hh]
                    nc.vector.scalar_tensor_tensor(
                        out=v, in0=C[l][:, off:off + hh],
                        scalar=float(2.0 ** (-t - EL[l])),
                        in1=A_[l][:, off:off + hh], op0=Alu.mult, op1=Alu.add)
                    if l == 0:
                        nc.vector.scalar_tensor_tensor(
                            out=v, in0=C0b[:, off:off + hh],
                            scalar=float(2.0 ** -(t + 11)),
                            in1=v, op0=Alu.mult, op1=Alu.add)

            def lif_spike(l, t, s_out, off, hh):
                """Spike threshold for one half (chain-critical)."""
                s_scale = 1.0 if l == 2 else float(2.0 ** t)
                nc.vector.tensor_scalar(
                    out=s_out[:, off:off + hh], in0=vT[l][:, off:off + hh],
                    scalar1=0.5, scalar2=s_scale, op0=Alu.is_gt, op1=Alu.mult)

            def lif_states(l, t, s_out, last):
                """State updates for step t+1 (off critical path)."""
                h = HS[l]
                v = vT[l][:, :h]
                v0, u0 = st[(l, "v0")], st[(l, "u0")]
                s_scale = 1.0 if l == 2 else float(2.0 ** t)
                if last:
                    return
                U = U_[l][:, :h]
                nc.vector.scalar_tensor_tensor(
                    out=U, in0=v0[:], scalar=float(-0.172 / 1.529), in1=u0[:],
                    op0=Alu.mult, op1=Alu.add)
                nc.scalar.mul(U, U, 1.529)
                nc.vector.scalar_tensor_tensor(
                    out=u0[:], in0=s_out[:], scalar=float(0.132 / s_scale),
                    in1=U, op0=Alu.mult, op1=Alu.add)
                nc.scalar.copy(v0[:], v)
                nc.vector.copy_predicated(out=v0[:],
                                          mask=s_out[:].bitcast(dt.uint16),
                                          data=c021[:, :h])

            def make_planes_half(s0, sP, off, hh):
                """4 fp8e5 scaled copies of one half of the L0 spikes."""
                nc.scalar.copy(sP[:, off:off + hh], s0[:, off:off + hh])
                nc.vector.tensor_scalar(
                    out=sP[:, H0 + off:H0 + off + hh], in0=s0[:, off:off + hh],
                    scalar1=float(2.0 ** -8), scalar2=None, op0=Alu.mult)
                nc.scalar.mul(sP[:, 2 * H0 + off:2 * H0 + off + hh],
                              s0[:, off:off + hh], float(2.0 ** -16))
                nc.scalar.mul(sP[:, 3 * H0 + off:3 * H0 + off + hh],
                              s0[:, off:off + hh], float(2.0 ** -16))

            sP_cur = [None]

            def emit_rest(t, filler=None):
                last = (t == T - 1)
                s0 = kp.tile([128, H0], F16, tag="s0", name=f"s0_t{t}")
                sP = kp.tile([128, 4 * H0], E5, tag="sP0", name=f"sP0_t{t}")
                sP_cur[0] = sP
                if last:
                    for off in (0, H0 // 2):
                        lif_spike(0, t, s0, off, H0 // 2)
                        make_planes_half(s0, sP, off, H0 // 2)
                else:
                    lif_spike(0, t, s0, 0, H0)
                    lif_states(0, t, s0, last)
                    lif_pre(0, t + 1)
                    make_planes_half(s0, sP, 0, H0)
                emit_dr(t)
                lif_release(1, t, halves=2 if last else 1)
                if filler is not None:
                    filler()
                s1 = kp.tile([128, H1], F16, tag="s1", name=f"s1_t{t}")
                s1L = kp.tile([128, H1], F16, tag="s1L", name=f"s1L_t{t}")
                if last:
                    for off in (0, H1 // 2):
                        lif_spike(1, t, s1, off, H1 // 2)
                        nc.scalar.mul(s1L[:, off:off + H1 // 2],
                                      s1[:, off:off + H1 // 2],
                                      float(2.0 ** -11))
                else:
                    lif_spike(1, t, s1, 0, H1)
                    nc.scalar.mul(s1L[:], s1[:], float(2.0 ** -11))
                    lif_states(1, t, s1, last)
                    lif_pre(1, t + 1)
                # L2: fp16 hi/lo 2-term (stationary = wo chunks); all hi terms
                # first so the PE has work before s1L lands
                for term in (0, 1):
                    for k in range(KT2):
                        ra = s1[:, k * 128:(k + 1) * 128]
                        rl = s1L[:, k * 128:(k + 1) * 128]
                        for m in range(MT2):
                            first = (term == 0 and t == 0 and k == 0 and
                                     (m * 128) % ZR == 0)
                            lastm = (term == 1 and t == T - 1 and k == KT2 - 1)
                            lwa = woa[:, k * OUT + m * 128:
                                      k * OUT + (m + 1) * 128]
                            lwl = wol[:, k * OUT + m * 128:
                                      k * OUT + (m + 1) * 128]
                            ps = C[2][:, m * 128:(m + 1) * 128]
                            if term == 0:
                                nc.tensor.matmul(ps, lwa, ra, start=first,
                                                 stop=False,
                                                 skip_group_check=True)
                            else:
                                nc.tensor.matmul(ps, lwl, rl, start=False,
                                                 stop=lastm,
                                                 skip_group_check=True)
                if t == 0:
                    inject_bias(2)
                lif_release(2, t)
                nc.vector.scalar_tensor_tensor(
                    out=acc[:], in0=vT[2][:, :OUT], scalar=0.5, in1=acc[:],
                    op0=Alu.is_gt, op1=Alu.add)
                if not last:
                    s2 = kp.tile([128, OUT], F16, tag="s2", name=f"s2_t{t}")
                    lif_spike(2, t, s2, 0, OUT)
                    lif_states(2, t, s2, last)
                    lif_pre(2, t + 1)

            # ---- main loop: 1-step layer skew ----
            for t in range(T):
                if t >= 1:
                    lif_release(0, t - 1)   # frees C0/C0b for step t's matmuls
                emit_L0(t, cis=(0,))
                if t == 0:
                    load_x(1, cis=(0,))
                    dma_w1d(range(2, 5))
                    load_x(1, cis=(1,))
                    dma_w1d(range(5, KT1))
                    dma_wo()
                    build_B2b()
                    for l in (0, 1, 2):
                        lif_pre(l, 0)
                    emit_L0(0, cis=(1,))
                else:
                    emit_rest(t - 1, filler=lambda tt=t: emit_L0(tt, cis=(1,)))
                    if t + 1 < T:
                        load_x(t + 1)
            lif_release(0, T - 1, halves=2)
            emit_rest(T - 1)

            nc.sync.dma_start(out=out_d[:], in_=acc[:])

    nc.compile()
    _BUILD_CACHE[key] = nc
    return nc


def _split_f16(a32, lo_scale=2048.0):
    hi = a32.astype(np.float16)
    lo = ((a32 - hi.astype(np.float32)) * np.float32(lo_scale)).astype(np.float16)
    return hi, lo


def _digit_planes(WT, ndig=NDIG, ew=EW):
    """WT [in,out] fp32 -> [in, ndig*out] fp8e4 digit planes (folded)."""
    r = WT.astype(np.float64) * (2.0 ** ew)
    assert np.max(np.abs(r)) <= 0.5, "weight exponent EW too small"
    planes = []
    for i in range(1, ndig + 1):
        di = np.rint(r * 16.0 ** i)
        di = np.clip(di, -4, 4) if i == ndig else np.clip(di, -8, 8)
        r = r - di * 16.0 ** -i
        planes.append(di * 2.0 ** -WFOLD[i - 1])
    out = np.concatenate(planes, axis=1).astype(ml_dtypes.float8_e4m3fn)
    assert np.all(out.astype(np.float64) == np.concatenate(planes, axis=1)), \
        "digit planes not exact in fp8e4"
    return out


def prep_inputs(in_pop_spikes, W0, b0, W1, b1, Wout, bout,
                T=16, BL=128, ncores=NCORES):
    x = np.ascontiguousarray(np.transpose(np.asarray(in_pop_spikes, np.float32),
                                          (2, 1, 0)))  # [T, IN, B]
    B = x.shape[2]
    IN = x.shape[1]
    scale = (2.0 ** np.arange(T, dtype=np.float32)).reshape(T, 1, 1)
    xh32 = x.astype(np.float16).astype(np.float32)
    xa = (xh32 * scale).astype(np.float16)
    xr = ((x - xh32) * (scale * np.float32(2048.0))).astype(np.float16)
    xz = np.stack([xa.reshape(T, IN // 128, 128, B),
                   xr.reshape(T, IN // 128, 128, B)], axis=3)
    xz = np.ascontiguousarray(xz.reshape(T, IN * 2, B))

    com = {}
    W0T = np.ascontiguousarray(np.asarray(W0, np.float32).T)
    com["w0a"], com["w0l"] = _split_f16(W0T)
    com["w1d"] = _digit_planes(np.ascontiguousarray(np.asarray(W1, np.float32).T))
    WoT = np.ascontiguousarray(np.asarray(Wout, np.float32).T)
    com["woa"], com["wol"] = _split_f16(WoT)
    for nm, b in (("br0", b0), ("br1", b1), ("br2", bout)):
        hi, lo = _split_f16(np.asarray(b, np.float32))
        com[nm] = np.stack([hi, lo])
    com["cpos"] = np.stack([np.full(128, 2.0, np.float16),
                            np.full(128, 2.0 / 2048.0, np.float16)])
    for l in (0, 1, 2):
        e = 2.0 ** EW if l == 1 else 1.0
        com[f"cneg{l}"] = np.stack([np.full(128, -e, np.float16),
                                    np.full(128, -e / 2048.0, np.float16)])

    in_maps = []
    for c in range(ncores):
        m = dict(com)
        m["xz"] = np.ascontiguousarray(xz[:, :, c * BL:(c + 1) * BL])
        in_maps.append(m)
    return in_maps


def kernel(in_pop_spikes, W0, b0, W1, b1, Wout, bout, batch_size, _trace=False):
    T = in_pop_spikes.shape[2]
    OUT, BL = Wout.shape[0], 128
    nc = build(**FULL)
    in_maps = prep_inputs(in_pop_spikes, W0, b0, W1, b1, Wout, bout, T=T)
    res = run_bass_kernel_spmd(nc, in_maps, core_ids=list(range(NCORES)),
                               trace=_trace)
    # device out[p, (c,b)] -> [b, c*128+p]
    outs = []
    for r in res.results:
        a = r["out"].reshape(128, OUT // 128, BL)
        outs.append(np.transpose(a, (2, 1, 0)).reshape(BL, OUT))
    out = (np.concatenate(outs, axis=0) / np.float32(T)).astype(np.float32)
    if _trace:
        kernel._last_results = res
    return out
